# revision 48
# baseline (speedup 1.0000x reference)
"""Trainium2 Bass kernel for nn_MoEFeedForward (top-2 MoE FFN, E=8 experts).

Strategy: expert-parallel across the 8 NeuronCores. The host computes the
routing metadata (gate logits -> top-2 expert ids) in fp32, gathers each
expert's routed tokens into a fixed-capacity buffer, and core e runs expert
e's FFN over its routed tokens in bf16 with fp32 accumulation:

    h   = gelu(x @ W1[e].T + b1[e])          (PSUM partition = F-chunk)
    out = w * (h @ W2[e].T + b2[e])          (PSUM partition = token)

The device also computes the gating network (bf16 hi/lo split matmul, exact
to ~1e-6), softmax probs and top-2 one-hot masks over a 512-token shard per
core, and reduces them to per-expert partial sums for the load-balancing aux
loss. The host unpermutes the pair outputs (every token has exactly 2 pairs)
and sums, and finishes the aux loss from the 16 partial sums per core.
"""

import contextlib
import ctypes
import os
import sys
import types

import numpy as np

try:
    import concourse.bass as bass  # noqa: F401
except ImportError:  # pragma: no cover
    sys.path.insert(0, "/opt/trn_rl_repo")

import ml_dtypes

import concourse.bacc as bacc
import concourse.mybir as mybir
import concourse.tile as tile
from concourse.bass_utils import run_bass_kernel_spmd

BF16 = ml_dtypes.bfloat16
P = 128
B, S, D, F, E = 2, 2048, 1024, 4096, 8
N = B * S  # 4096 tokens
TOKENS_PER_CORE = N // E  # 512, gating shard per core
LB_COEF = 0.01
C_DEFAULT = 1152  # routed-pair capacity per expert (mean load is 1024)
C_LADDER = [1152, 1280, 1408, 1536, 2048, 4096]

_BF = mybir.dt.bfloat16
_F32 = mybir.dt.float32
_AF = mybir.ActivationFunctionType
_ALU = mybir.AluOpType
_AX = mybir.AxisListType


def _install_ntff_hook():
    """Make run_bass_kernel_spmd(trace=True) usable under axon: supply the
    antenv.axon_hooks module the container image lacks. Harmless when tracing
    is never requested; required because BASS_TRACE=1 in the environment
    would otherwise crash on the missing import."""
    if "antenv.axon_hooks" in sys.modules:
        return

    def _make(so_path):
        try:
            lib = ctypes.CDLL(so_path)
        except OSError:
            return None
        if not hasattr(lib, "axon_start_nrt_profile"):
            return None
        lib.axon_start_nrt_profile.argtypes = [
            ctypes.POINTER(ctypes.c_int64),
            ctypes.c_size_t,
        ]
        lib.axon_start_nrt_profile.restype = ctypes.c_int64
        lib.axon_stop_nrt_profile.argtypes = [ctypes.c_char_p]
        lib.axon_stop_nrt_profile.restype = ctypes.c_int64

        @contextlib.contextmanager
        def _hook(output_dir, device_ids):
            import jax

            jax.devices()
            if device_ids:
                ids = (ctypes.c_int64 * len(device_ids))(*device_ids)
                rc = lib.axon_start_nrt_profile(ids, len(device_ids))
            else:
                rc = lib.axon_start_nrt_profile(None, 0)
            if rc != 0:
                raise RuntimeError(f"axon_start_nrt_profile rc={rc}")
            try:
                yield
            finally:
                n = lib.axon_stop_nrt_profile(str(output_dir).encode())
                print(f"profile: {n} file(s) written to {output_dir}", file=sys.stderr)

        return _hook

    hook = _make("/opt/axon/libaxon_pjrt.so")
    mod = types.ModuleType("antenv.axon_hooks")
    mod.get_axon_ntff_profile_hook = lambda: hook
    mod.set_axon_ntff_profile_hook = lambda h: None
    sys.modules["antenv.axon_hooks"] = mod


_install_ntff_hook()





def _build(C):
    """Build + compile the per-core Bass program for pair capacity C."""
    assert C % P == 0
    NF = F // P  # 32 f-chunks
    NDO = D // P  # 8 d-chunks
    NTS = C // P  # token-subtiles of 128 pairs
    NDQ = D // 256  # 4 output quarters of 256
    TG = TOKENS_PER_CORE  # 512 gating tokens per core
    NTT = TG // P  # 4 gating token-subtiles

    nc = bacc.Bacc("TRN2", target_bir_lowering=False, debug=False)

    xp = nc.dram_tensor("xp", [P, NDO, C], _BF, kind="ExternalInput")
    w1t = nc.dram_tensor("w1t", [NF // 2, P, 2, NDO, P], _BF, kind="ExternalInput")
    w2t = nc.dram_tensor("w2t", [NDQ, P, NF, 256], _BF, kind="ExternalInput")
    b1c = nc.dram_tensor("b1c", [P, NF], _F32, kind="ExternalInput")
    b2b = nc.dram_tensor("b2b", [P, D], _F32, kind="ExternalInput")
    wp = nc.dram_tensor("wp", [P, NTS], _F32, kind="ExternalInput")
    xg_hi = nc.dram_tensor("xg_hi", [P, NDO, TG], _BF, kind="ExternalInput")
    xg_lo = nc.dram_tensor("xg_lo", [P, NDO, TG], _BF, kind="ExternalInput")
    wg_hi = nc.dram_tensor("wg_hi", [P, NDO, E], _BF, kind="ExternalInput")
    wg_lo = nc.dram_tensor("wg_lo", [P, NDO, E], _BF, kind="ExternalInput")
    out = nc.dram_tensor("out", [C, D], _F32, kind="ExternalOutput")
    stats = nc.dram_tensor("stats", [1, 2 * E], _F32, kind="ExternalOutput")

    with tile.TileContext(nc) as tc:
        with (
            tc.tile_pool(name="const", bufs=1) as const,
            tc.tile_pool(name="w1p", bufs=6) as w1p,
            tc.tile_pool(name="w2p", bufs=2) as w2p,
            tc.tile_pool(name="hp", bufs=1) as hp,
            tc.tile_pool(name="evp", bufs=4) as evp,
            tc.tile_pool(name="ps1", bufs=3, space="PSUM") as ps1,
            tc.tile_pool(name="ps2", bufs=2, space="PSUM") as ps2,
        ):
            # Dependency-free warmup matmuls: ~3.5us of PE activity at t=0
            # flips the HAM clock gate to 2.4 GHz while the first DMAs land.
            warm_in = const.tile([P, 512], _BF, tag="warm_in")
            nc.vector.memset(warm_in[:], 0.0)
            for _ in range(22):
                wps = ps1.tile([P, 512], _F32, tag="pm1")
                nc.tensor.matmul(wps[:], warm_in[:, :P], warm_in[:], start=True, stop=True)

            # W1 streams in 512KB pair-of-f-tiles alternating between the
            # sync and scalar HWDGE queues (one queue alone is marginal
            # against mm1's ~67 GB/s consumption). Prefetch the first two.
            N_PRE = 2
            w1_pre = []
            for fp_ in range(N_PRE):
                t = w1p.tile([P, 2, NDO, P], _BF, tag="w1tile")
                (nc.sync if fp_ % 2 == 0 else nc.scalar).dma_start(t[:], w1t[fp_])
                w1_pre.append(t)

            # Pair activations: per-d-chunk DMAs split across both HWDGE
            # queues right behind the first W1 pair, so mm1's first PSUM
            # group has all 8 chunks within ~8us.
            xp_sb = const.tile([P, NDO, C], _BF)
            for do in range(NDO):
                nc.gpsimd.dma_start(xp_sb[:, do : do + 1, :], xp[:, do : do + 1, :])

            # Bulk prefetches on the gpsimd DMA queue (decoupled from the
            # latency-critical sync queue).
            b1_sb = const.tile([P, NF], _F32)
            nc.gpsimd.dma_start(b1_sb[:], b1c[:])
            b2b_sb = const.tile([P, D], _F32)
            nc.gpsimd.dma_start(b2b_sb[:], b2b[:])
            wp_sb = const.tile([P, NTS], _F32)
            nc.gpsimd.dma_start(wp_sb[:], wp[:])
            ones_sb = const.tile([P, 1], _BF)
            nc.vector.memset(ones_sb[:], 1.0)
            xgh_sb = const.tile([P, NDO, TG], _BF, tag="xgh")
            nc.gpsimd.dma_start(xgh_sb[:], xg_hi[:])
            xgl_sb = const.tile([P, NDO, TG], _BF, tag="xgl")
            nc.gpsimd.dma_start(xgl_sb[:], xg_lo[:])
            wgh_sb = const.tile([P, NDO, E], _BF, tag="wgh")
            nc.gpsimd.dma_start(wgh_sb[:], wg_hi[:])
            wgl_sb = const.tile([P, NDO, E], _BF, tag="wgl")
            nc.gpsimd.dma_start(wgl_sb[:], wg_lo[:])

            hT = hp.tile([P, NF, C], _BF)
            blocks = []
            t0 = 0
            while t0 < C:
                tn = min(512, C - t0)
                blocks.append((t0, tn))
                t0 += tn

            # ---- mm1: hT[f, t] = gelu(sum_d W1T[d, f] * x[d, t] + b1[f]) ----
            for fp_ in range(NF // 2):
                if fp_ < N_PRE:
                    w1tile = w1_pre[fp_]
                else:
                    w1tile = w1p.tile([P, 2, NDO, P], _BF, tag="w1tile")
                    (nc.sync if fp_ % 2 == 0 else nc.scalar).dma_start(
                        w1tile[:], w1t[fp_]
                    )
                for j in range(2):
                    ft = fp_ * 2 + j
                    for t0, tn in blocks:
                        pm = ps1.tile([P, 512], _F32, tag="pm1")
                        for do in range(NDO):
                            nc.tensor.matmul(
                                pm[:, :tn],
                                w1tile[:, j, do, :],
                                xp_sb[:, do, t0 : t0 + tn],
                                start=(do == 0),
                                stop=(do == NDO - 1),
                            )
                        nc.scalar.activation(
                            hT[:, ft, t0 : t0 + tn],
                            pm[:, :tn],
                            _AF.Gelu,
                            bias=b1_sb[:, ft : ft + 1],
                        )

            # ---- mm2: out[t, dd] = w[t] * (sum_f hT[f, t]*W2T[f, dd] + b2[dd]) ----
            for dq in range(NDQ):
                w2q = w2p.tile([P, NF, 256], _BF, tag="w2q")
                nc.gpsimd.dma_start(w2q[:], w2t[dq])
                for ts in range(NTS):
                    pm2 = ps2.tile([P, 256], _F32, tag="pm2")
                    for ft in range(NF):
                        nc.tensor.matmul(
                            pm2[:],
                            hT[:, ft, ts * P : (ts + 1) * P],
                            w2q[:, ft, :],
                            start=(ft == 0),
                            stop=(ft == NF - 1),
                        )
                    tmp = evp.tile([P, 256], _F32, tag="ev_tmp")
                    nc.vector.tensor_add(
                        tmp[:], pm2[:], b2b_sb[:, dq * 256 : (dq + 1) * 256]
                    )
                    ot = evp.tile([P, 256], _F32, tag="ev_out")
                    nc.scalar.activation(
                        ot[:], tmp[:], _AF.Copy, scale=wp_sb[:, ts : ts + 1]
                    )
                    nc.sync.dma_start(
                        out[ts * P : (ts + 1) * P, dq * 256 : (dq + 1) * 256],
                        ot[:],
                    )

            # ---- gating + aux-loss partial sums (PE work here fills the
            # post-mm2 eviction/drain tail) ----
            with (
                tc.tile_pool(name="gtmp", bufs=2) as gtmp,
                tc.tile_pool(name="gps", bufs=1, space="PSUM") as gps,
                tc.tile_pool(name="sps", bufs=1, space="PSUM") as sps,
            ):
                ps_load = sps.tile([1, E], _F32, tag="psload")
                ps_imp = sps.tile([1, E], _F32, tag="psimp")

                for tt in range(NTT):
                    pg = gps.tile([P, E], _F32, tag="pg")
                    terms = [(xgh_sb, wgh_sb), (xgh_sb, wgl_sb), (xgl_sb, wgh_sb)]
                    k = 0
                    for xa, wa in terms:
                        for do in range(NDO):
                            nc.tensor.matmul(
                                pg[:],
                                xa[:, do, tt * P : (tt + 1) * P],
                                wa[:, do, :],
                                start=(k == 0),
                                stop=(k == 3 * NDO - 1),
                            )
                            k += 1
                    lg = gtmp.tile([P, E], _F32, tag="lg")
                    nc.vector.tensor_copy(lg[:], pg[:])

                    # softmax over the free (expert) axis
                    m1 = gtmp.tile([P, 1], _F32, tag="m1")
                    nc.vector.reduce_max(m1[:], lg[:], axis=_AX.X)
                    sh = gtmp.tile([P, E], _F32, tag="sh")
                    nc.vector.tensor_sub(sh[:], lg[:], m1[:].to_broadcast((P, E)))
                    ex = gtmp.tile([P, E], _F32, tag="ex")
                    nc.scalar.activation(ex[:], sh[:], _AF.Exp)
                    ssum = gtmp.tile([P, 1], _F32, tag="ssum")
                    nc.vector.reduce_sum(ssum[:], ex[:], axis=_AX.X)
                    rec = gtmp.tile([P, 1], _F32, tag="rec")
                    nc.vector.reciprocal(rec[:], ssum[:])
                    pr = gtmp.tile([P, E], _F32, tag="pr")
                    nc.vector.tensor_mul(pr[:], ex[:], rec[:].to_broadcast((P, E)))

                    # top-2 one-hot masks
                    is1 = gtmp.tile([P, E], _F32, tag="is1")
                    nc.vector.tensor_tensor(
                        is1[:], lg[:], m1[:].to_broadcast((P, E)), _ALU.is_equal
                    )
                    pen = gtmp.tile([P, E], _F32, tag="pen")
                    nc.vector.tensor_scalar_mul(pen[:], is1[:], 1e30)
                    msk = gtmp.tile([P, E], _F32, tag="msk")
                    nc.vector.tensor_sub(msk[:], lg[:], pen[:])
                    m2 = gtmp.tile([P, 1], _F32, tag="m2")
                    nc.vector.reduce_max(m2[:], msk[:], axis=_AX.X)
                    is2 = gtmp.tile([P, E], _F32, tag="is2")
                    nc.vector.tensor_tensor(
                        is2[:], msk[:], m2[:].to_broadcast((P, E)), _ALU.is_equal
                    )
                    s12 = gtmp.tile([P, E], _F32, tag="s12")
                    nc.vector.tensor_add(s12[:], is1[:], is2[:])
                    s12b = gtmp.tile([P, E], _BF, tag="s12b")
                    nc.vector.tensor_copy(s12b[:], s12[:])

                    # probs in bf16 hi/lo so the ones-matmul stays exact
                    prh = gtmp.tile([P, E], _BF, tag="prh")
                    nc.vector.tensor_copy(prh[:], pr[:])
                    prhf = gtmp.tile([P, E], _F32, tag="prhf")
                    nc.vector.tensor_copy(prhf[:], prh[:])
                    prlf = gtmp.tile([P, E], _F32, tag="prlf")
                    nc.vector.tensor_sub(prlf[:], pr[:], prhf[:])
                    prl = gtmp.tile([P, E], _BF, tag="prl")
                    nc.vector.tensor_copy(prl[:], prlf[:])

                    nc.tensor.matmul(
                        ps_load[:], ones_sb[:], s12b[:],
                        start=(tt == 0), stop=(tt == NTT - 1),
                    )
                    nc.tensor.matmul(
                        ps_imp[:], ones_sb[:], prh[:],
                        start=(tt == 0), stop=False,
                    )
                    nc.tensor.matmul(
                        ps_imp[:], ones_sb[:], prl[:],
                        start=False, stop=(tt == NTT - 1),
                    )

                st = gtmp.tile([1, 2 * E], _F32, tag="st")
                nc.vector.tensor_copy(st[:, 0:E], ps_load[:])
                nc.vector.tensor_copy(st[:, E : 2 * E], ps_imp[:])
                nc.sync.dma_start(stats[:], st[:])

    nc.compile()
    return nc


_compiled = {}


def _get_nc(C):
    if C not in _compiled:
        _compiled[C] = _build(C)
    return _compiled[C]


def _dp_major(a, inner=P):
    """[K, M] -> [inner, K//inner, M] with the K axis innermost-major on
    partitions: out[p, o, m] = a[o*inner + p, m]."""
    k, m = a.shape
    return np.ascontiguousarray(a.reshape(k // inner, inner, m).transpose(1, 0, 2))


def kernel(x, Wg, W1, b1, W2, b2):
    x = np.asarray(x, dtype=np.float32)
    Wg = np.asarray(Wg, dtype=np.float32)
    W1 = np.asarray(W1, dtype=np.float32)
    b1 = np.asarray(b1, dtype=np.float32)
    W2 = np.asarray(W2, dtype=np.float32)
    b2 = np.asarray(b2, dtype=np.float32)
    assert x.shape == (B, S, D) and W1.shape == (E, F, D) and W2.shape == (E, D, F)

    x2d = x.reshape(N, D)

    # ---- host routing metadata (fp32, matches jax.lax.top_k tie-breaking) ----
    logits = x2d @ Wg.T  # [N, E]
    i1 = logits.argmax(1)
    masked = logits.copy()
    masked[np.arange(N), i1] = -np.inf
    i2 = masked.argmax(1)
    l1 = logits[np.arange(N), i1]
    l2 = logits[np.arange(N), i2]
    e2 = np.exp((l2 - l1).astype(np.float32))
    wt1 = (1.0 / (1.0 + e2)).astype(np.float32)
    wt2 = (e2 / (1.0 + e2)).astype(np.float32)

    tok_lists, wt_lists, counts = [], [], []
    for e in range(E):
        t_a = np.nonzero(i1 == e)[0]
        t_b = np.nonzero(i2 == e)[0]
        tok_lists.append(np.concatenate([t_a, t_b]))
        wt_lists.append(np.concatenate([wt1[t_a], wt2[t_b]]).astype(np.float32))
        counts.append(len(t_a) + len(t_b))
    max_load = max(counts)

    C = next((c for c in C_LADDER if c >= max_load), None)
    if C is None:
        C = ((max_load + P - 1) // P) * P
    nc = _get_nc(C)
    NTS = C // P

    # ---- per-core inputs ----
    WgT = Wg.T  # [D, E]
    wg_hi = WgT.astype(BF16)
    wg_lo = (WgT - wg_hi.astype(np.float32)).astype(BF16)
    wg_hi_l = _dp_major(wg_hi.astype(np.float32)).astype(BF16)
    wg_lo_l = _dp_major(wg_lo.astype(np.float32)).astype(BF16)

    in_maps = []
    for e in range(E):
        toks = np.zeros(C, dtype=np.int64)
        toks[: counts[e]] = tok_lists[e]
        wts = np.zeros(C, dtype=np.float32)
        wts[: counts[e]] = wt_lists[e]

        xpair = x2d[toks]  # [C, D]
        xp_l = _dp_major(xpair.T.astype(BF16).astype(np.float32)).astype(BF16)

        w1e = W1[e].reshape(F // P, P, D // P, P)  # [ft, fi, do, dp]
        w1t_l = (
            w1e.transpose(0, 3, 2, 1)  # [ft, dp, do, fi]
            .reshape(F // (2 * P), 2, P, D // P, P)  # [fp, j, dp, do, fi]
            .transpose(0, 2, 1, 3, 4)  # [fp, dp, j, do, fi]
        )
        w1t_l = np.ascontiguousarray(w1t_l).astype(BF16)
        w2e = W2[e].reshape(D // 256, 256, F // P, P)  # [dq, j, ft, fp]
        w2t_l = np.ascontiguousarray(w2e.transpose(0, 3, 2, 1)).astype(BF16)

        b1_l = np.ascontiguousarray(b1[e].reshape(F // P, P).T)  # [P, NF]
        b2b_l = np.ascontiguousarray(np.broadcast_to(b2[e], (P, D)))
        wp_l = np.ascontiguousarray(wts.reshape(NTS, P).T)  # [P, NTS]

        sl = slice(e * TOKENS_PER_CORE, (e + 1) * TOKENS_PER_CORE)
        xg = x2d[sl].T  # [D, TG] fp32
        xg_hi = xg.astype(BF16)
        xg_lo = (xg - xg_hi.astype(np.float32)).astype(BF16)
        xg_hi_l = _dp_major(xg_hi.astype(np.float32)).astype(BF16)
        xg_lo_l = _dp_major(xg_lo.astype(np.float32)).astype(BF16)

        in_maps.append(
            {
                "xp": xp_l,
                "w1t": w1t_l,
                "w2t": w2t_l,
                "b1c": b1_l,
                "b2b": b2b_l,
                "wp": wp_l,
                "xg_hi": xg_hi_l,
                "xg_lo": xg_lo_l,
                "wg_hi": wg_hi_l,
                "wg_lo": wg_lo_l,
            }
        )

    res = run_bass_kernel_spmd(nc, in_maps, core_ids=list(range(E)))

    # ---- host combine: unpermute pairs, sum the 2 pairs per token ----
    pair_out = np.concatenate(
        [res.results[e]["out"][: counts[e]] for e in range(E)], axis=0
    )  # [2N, D], already weighted + biased
    pair_tok = np.concatenate([tok_lists[e] for e in range(E)])
    order = np.argsort(pair_tok, kind="stable")
    out2d = pair_out[order].reshape(N, 2, D).sum(axis=1)

    # ---- aux loss from device partial sums ----
    stats = np.stack([res.results[e]["stats"][0] for e in range(E)])  # [E, 2E]
    load = stats[:, :E].sum(0) / N
    imp = stats[:, E:].sum(0) / N
    aux = np.float32(LB_COEF * E * np.sum(load * imp))

    return out2d.reshape(B, S, D).astype(np.float32), aux


# revision 50
# speedup vs baseline: 1.0130x; 1.0130x over previous
"""Trainium2 Bass kernel for nn_MoEFeedForward (top-2 MoE FFN, E=8 experts).

Strategy: expert-parallel across the 8 NeuronCores. The host computes the
routing metadata (gate logits -> top-2 expert ids) in fp32, gathers each
expert's routed tokens into a fixed-capacity buffer, and core e runs expert
e's FFN over its routed tokens in bf16 with fp32 accumulation:

    h   = gelu(x @ W1[e].T + b1[e])          (PSUM partition = F-chunk)
    out = w * (h @ W2[e].T + b2[e])          (PSUM partition = token)

The device also computes the gating network (bf16 hi/lo split matmul, exact
to ~1e-6), softmax probs and top-2 one-hot masks over a 512-token shard per
core, and reduces them to per-expert partial sums for the load-balancing aux
loss. The host unpermutes the pair outputs (every token has exactly 2 pairs)
and sums, and finishes the aux loss from the 16 partial sums per core.
"""

import contextlib
import ctypes
import os
import sys
import types

import numpy as np

try:
    import concourse.bass as bass  # noqa: F401
except ImportError:  # pragma: no cover
    sys.path.insert(0, "/opt/trn_rl_repo")

import ml_dtypes

import concourse.bacc as bacc
import concourse.mybir as mybir
import concourse.tile as tile
from concourse.bass_utils import run_bass_kernel_spmd

BF16 = ml_dtypes.bfloat16
P = 128
B, S, D, F, E = 2, 2048, 1024, 4096, 8
N = B * S  # 4096 tokens
TOKENS_PER_CORE = N // E  # 512, gating shard per core
LB_COEF = 0.01
C_DEFAULT = 1152  # routed-pair capacity per expert (mean load is 1024)
C_LADDER = [1152, 1280, 1408, 1536, 2048, 4096]

_BF = mybir.dt.bfloat16
_F32 = mybir.dt.float32
_AF = mybir.ActivationFunctionType
_ALU = mybir.AluOpType
_AX = mybir.AxisListType


def _install_ntff_hook():
    """Make run_bass_kernel_spmd(trace=True) usable under axon: supply the
    antenv.axon_hooks module the container image lacks. Harmless when tracing
    is never requested; required because BASS_TRACE=1 in the environment
    would otherwise crash on the missing import."""
    if "antenv.axon_hooks" in sys.modules:
        return

    def _make(so_path):
        try:
            lib = ctypes.CDLL(so_path)
        except OSError:
            return None
        if not hasattr(lib, "axon_start_nrt_profile"):
            return None
        lib.axon_start_nrt_profile.argtypes = [
            ctypes.POINTER(ctypes.c_int64),
            ctypes.c_size_t,
        ]
        lib.axon_start_nrt_profile.restype = ctypes.c_int64
        lib.axon_stop_nrt_profile.argtypes = [ctypes.c_char_p]
        lib.axon_stop_nrt_profile.restype = ctypes.c_int64

        @contextlib.contextmanager
        def _hook(output_dir, device_ids):
            import jax

            jax.devices()
            if device_ids:
                ids = (ctypes.c_int64 * len(device_ids))(*device_ids)
                rc = lib.axon_start_nrt_profile(ids, len(device_ids))
            else:
                rc = lib.axon_start_nrt_profile(None, 0)
            if rc != 0:
                raise RuntimeError(f"axon_start_nrt_profile rc={rc}")
            try:
                yield
            finally:
                n = lib.axon_stop_nrt_profile(str(output_dir).encode())
                print(f"profile: {n} file(s) written to {output_dir}", file=sys.stderr)

        return _hook

    hook = _make("/opt/axon/libaxon_pjrt.so")
    mod = types.ModuleType("antenv.axon_hooks")
    mod.get_axon_ntff_profile_hook = lambda: hook
    mod.set_axon_ntff_profile_hook = lambda h: None
    sys.modules["antenv.axon_hooks"] = mod


_install_ntff_hook()





def _build(C):
    """Build + compile the per-core Bass program for pair capacity C."""
    assert C % P == 0
    NF = F // P  # 32 f-chunks
    NDO = D // P  # 8 d-chunks
    NTS = C // P  # token-subtiles of 128 pairs
    NDQ = D // 256  # 4 output quarters of 256
    TG = TOKENS_PER_CORE  # 512 gating tokens per core
    NTT = TG // P  # 4 gating token-subtiles

    nc = bacc.Bacc("TRN2", target_bir_lowering=False, debug=False)

    xp = nc.dram_tensor("xp", [P, NDO, C], _BF, kind="ExternalInput")
    w1t = nc.dram_tensor("w1t", [NF // 2, P, 2, NDO, P], _BF, kind="ExternalInput")
    w2t = nc.dram_tensor("w2t", [NDQ, P, NF, 256], _BF, kind="ExternalInput")
    b1c = nc.dram_tensor("b1c", [P, NF], _F32, kind="ExternalInput")
    b2b = nc.dram_tensor("b2b", [P, D], _F32, kind="ExternalInput")
    wp = nc.dram_tensor("wp", [P, NTS], _F32, kind="ExternalInput")
    xg_hi = nc.dram_tensor("xg_hi", [P, NDO, TG], _BF, kind="ExternalInput")
    xg_lo = nc.dram_tensor("xg_lo", [P, NDO, TG], _BF, kind="ExternalInput")
    wg_hi = nc.dram_tensor("wg_hi", [P, NDO, E], _BF, kind="ExternalInput")
    wg_lo = nc.dram_tensor("wg_lo", [P, NDO, E], _BF, kind="ExternalInput")
    out = nc.dram_tensor("out", [C, D], _F32, kind="ExternalOutput")
    stats = nc.dram_tensor("stats", [1, 2 * E], _F32, kind="ExternalOutput")

    with tile.TileContext(nc) as tc:
        with (
            tc.tile_pool(name="const", bufs=1) as const,
            tc.tile_pool(name="w1p", bufs=6) as w1p,
            tc.tile_pool(name="w2p", bufs=2) as w2p,
            tc.tile_pool(name="hp", bufs=1) as hp,
            tc.tile_pool(name="evp", bufs=4) as evp,
            tc.tile_pool(name="ps1", bufs=3, space="PSUM") as ps1,
            tc.tile_pool(name="ps2", bufs=2, space="PSUM") as ps2,
        ):
            # Dependency-free warmup matmuls: ~3.5us of PE activity at t=0
            # flips the HAM clock gate to 2.4 GHz while the first DMAs land.
            warm_in = const.tile([P, 512], _BF, tag="warm_in")
            nc.vector.memset(warm_in[:], 0.0)
            for _ in range(22):
                wps = ps1.tile([P, 512], _F32, tag="pm1")
                nc.tensor.matmul(wps[:], warm_in[:, :P], warm_in[:], start=True, stop=True)

            # W1 streams in 512KB pair-of-f-tiles alternating between the
            # sync and scalar HWDGE queues (one queue alone is marginal
            # against mm1's ~67 GB/s consumption). Prefetch the first two.
            N_PRE = 2
            w1_pre = []
            for fp_ in range(N_PRE):
                t = w1p.tile([P, 2, NDO, P], _BF, tag="w1tile")
                (nc.sync if fp_ % 2 == 0 else nc.scalar).dma_start(t[:], w1t[fp_])
                w1_pre.append(t)

            # Pair activations: per-d-chunk DMAs split across both HWDGE
            # queues right behind the first W1 pair, so mm1's first PSUM
            # group has all 8 chunks within ~8us.
            xp_sb = const.tile([P, NDO, C], _BF)
            for do in range(NDO):
                nc.gpsimd.dma_start(xp_sb[:, do : do + 1, :], xp[:, do : do + 1, :])

            # Bulk prefetches on the gpsimd DMA queue (decoupled from the
            # latency-critical sync queue).
            b1_sb = const.tile([P, NF], _F32)
            nc.gpsimd.dma_start(b1_sb[:], b1c[:])
            b2b_sb = const.tile([P, D], _F32)
            nc.gpsimd.dma_start(b2b_sb[:], b2b[:])
            wp_sb = const.tile([P, NTS], _F32)
            nc.gpsimd.dma_start(wp_sb[:], wp[:])
            ones_sb = const.tile([P, 1], _BF)
            nc.vector.memset(ones_sb[:], 1.0)
            xgh_sb = const.tile([P, NDO, TG], _BF, tag="xgh")
            nc.gpsimd.dma_start(xgh_sb[:], xg_hi[:])
            xgl_sb = const.tile([P, NDO, TG], _BF, tag="xgl")
            nc.gpsimd.dma_start(xgl_sb[:], xg_lo[:])
            wgh_sb = const.tile([P, NDO, E], _BF, tag="wgh")
            nc.gpsimd.dma_start(wgh_sb[:], wg_hi[:])
            wgl_sb = const.tile([P, NDO, E], _BF, tag="wgl")
            nc.gpsimd.dma_start(wgl_sb[:], wg_lo[:])

            hT = hp.tile([P, NF, C], _BF)
            blocks = []
            t0 = 0
            while t0 < C:
                tn = min(512, C - t0)
                blocks.append((t0, tn))
                t0 += tn

            # ---- mm1: hT[f, t] = gelu(sum_d W1T[d, f] * x[d, t] + b1[f]) ----
            for fp_ in range(NF // 2):
                if fp_ < N_PRE:
                    w1tile = w1_pre[fp_]
                else:
                    w1tile = w1p.tile([P, 2, NDO, P], _BF, tag="w1tile")
                    (nc.sync if fp_ % 2 == 0 else nc.scalar).dma_start(
                        w1tile[:], w1t[fp_]
                    )
                for j in range(2):
                    ft = fp_ * 2 + j
                    for t0, tn in blocks:
                        pm = ps1.tile([P, 512], _F32, tag="pm1")
                        for do in range(NDO):
                            nc.tensor.matmul(
                                pm[:, :tn],
                                w1tile[:, j, do, :],
                                xp_sb[:, do, t0 : t0 + tn],
                                start=(do == 0),
                                stop=(do == NDO - 1),
                            )
                        nc.scalar.activation(
                            hT[:, ft, t0 : t0 + tn],
                            pm[:, :tn],
                            _AF.Gelu,
                            bias=b1_sb[:, ft : ft + 1],
                        )

            # ---- gating + aux-loss partial sums (PE work here overlaps the
            # tail gelu evictions mm2 must wait for) ----
            with (
                tc.tile_pool(name="gtmp", bufs=2) as gtmp,
                tc.tile_pool(name="gps", bufs=1, space="PSUM") as gps,
                tc.tile_pool(name="sps", bufs=1, space="PSUM") as sps,
            ):
                ps_load = sps.tile([1, E], _F32, tag="psload")
                ps_imp = sps.tile([1, E], _F32, tag="psimp")

                for tt in range(NTT):
                    pg = gps.tile([P, E], _F32, tag="pg")
                    terms = [(xgh_sb, wgh_sb), (xgh_sb, wgl_sb), (xgl_sb, wgh_sb)]
                    k = 0
                    for xa, wa in terms:
                        for do in range(NDO):
                            nc.tensor.matmul(
                                pg[:],
                                xa[:, do, tt * P : (tt + 1) * P],
                                wa[:, do, :],
                                start=(k == 0),
                                stop=(k == 3 * NDO - 1),
                            )
                            k += 1
                    lg = gtmp.tile([P, E], _F32, tag="lg")
                    nc.vector.tensor_copy(lg[:], pg[:])

                    # softmax over the free (expert) axis
                    m1 = gtmp.tile([P, 1], _F32, tag="m1")
                    nc.vector.reduce_max(m1[:], lg[:], axis=_AX.X)
                    sh = gtmp.tile([P, E], _F32, tag="sh")
                    nc.vector.tensor_sub(sh[:], lg[:], m1[:].to_broadcast((P, E)))
                    ex = gtmp.tile([P, E], _F32, tag="ex")
                    nc.scalar.activation(ex[:], sh[:], _AF.Exp)
                    ssum = gtmp.tile([P, 1], _F32, tag="ssum")
                    nc.vector.reduce_sum(ssum[:], ex[:], axis=_AX.X)
                    rec = gtmp.tile([P, 1], _F32, tag="rec")
                    nc.vector.reciprocal(rec[:], ssum[:])
                    pr = gtmp.tile([P, E], _F32, tag="pr")
                    nc.vector.tensor_mul(pr[:], ex[:], rec[:].to_broadcast((P, E)))

                    # top-2 one-hot masks
                    is1 = gtmp.tile([P, E], _F32, tag="is1")
                    nc.vector.tensor_tensor(
                        is1[:], lg[:], m1[:].to_broadcast((P, E)), _ALU.is_equal
                    )
                    pen = gtmp.tile([P, E], _F32, tag="pen")
                    nc.vector.tensor_scalar_mul(pen[:], is1[:], 1e30)
                    msk = gtmp.tile([P, E], _F32, tag="msk")
                    nc.vector.tensor_sub(msk[:], lg[:], pen[:])
                    m2 = gtmp.tile([P, 1], _F32, tag="m2")
                    nc.vector.reduce_max(m2[:], msk[:], axis=_AX.X)
                    is2 = gtmp.tile([P, E], _F32, tag="is2")
                    nc.vector.tensor_tensor(
                        is2[:], msk[:], m2[:].to_broadcast((P, E)), _ALU.is_equal
                    )
                    s12 = gtmp.tile([P, E], _F32, tag="s12")
                    nc.vector.tensor_add(s12[:], is1[:], is2[:])
                    s12b = gtmp.tile([P, E], _BF, tag="s12b")
                    nc.vector.tensor_copy(s12b[:], s12[:])

                    # probs in bf16 hi/lo so the ones-matmul stays exact
                    prh = gtmp.tile([P, E], _BF, tag="prh")
                    nc.vector.tensor_copy(prh[:], pr[:])
                    prhf = gtmp.tile([P, E], _F32, tag="prhf")
                    nc.vector.tensor_copy(prhf[:], prh[:])
                    prlf = gtmp.tile([P, E], _F32, tag="prlf")
                    nc.vector.tensor_sub(prlf[:], pr[:], prhf[:])
                    prl = gtmp.tile([P, E], _BF, tag="prl")
                    nc.vector.tensor_copy(prl[:], prlf[:])

                    nc.tensor.matmul(
                        ps_load[:], ones_sb[:], s12b[:],
                        start=(tt == 0), stop=(tt == NTT - 1),
                    )
                    nc.tensor.matmul(
                        ps_imp[:], ones_sb[:], prh[:],
                        start=(tt == 0), stop=False,
                    )
                    nc.tensor.matmul(
                        ps_imp[:], ones_sb[:], prl[:],
                        start=False, stop=(tt == NTT - 1),
                    )

                st = gtmp.tile([1, 2 * E], _F32, tag="st")
                nc.vector.tensor_copy(st[:, 0:E], ps_load[:])
                nc.vector.tensor_copy(st[:, E : 2 * E], ps_imp[:])
                nc.sync.dma_start(stats[:], st[:])

            # ---- mm2: out[t, dd] = w[t] * (sum_f hT[f, t]*W2T[f, dd] + b2[dd]) ----
            for dq in range(NDQ):
                w2q = w2p.tile([P, NF, 256], _BF, tag="w2q")
                nc.gpsimd.dma_start(w2q[:], w2t[dq])
                for ts in range(NTS):
                    pm2 = ps2.tile([P, 256], _F32, tag="pm2")
                    for ft in range(NF):
                        nc.tensor.matmul(
                            pm2[:],
                            hT[:, ft, ts * P : (ts + 1) * P],
                            w2q[:, ft, :],
                            start=(ft == 0),
                            stop=(ft == NF - 1),
                        )
                    tmp = evp.tile([P, 256], _F32, tag="ev_tmp")
                    nc.vector.tensor_add(
                        tmp[:], pm2[:], b2b_sb[:, dq * 256 : (dq + 1) * 256]
                    )
                    ot = evp.tile([P, 256], _F32, tag="ev_out")
                    nc.scalar.activation(
                        ot[:], tmp[:], _AF.Copy, scale=wp_sb[:, ts : ts + 1]
                    )
                    nc.sync.dma_start(
                        out[ts * P : (ts + 1) * P, dq * 256 : (dq + 1) * 256],
                        ot[:],
                    )

    nc.compile()
    return nc


_compiled = {}


def _get_nc(C):
    if C not in _compiled:
        _compiled[C] = _build(C)
    return _compiled[C]


def _dp_major(a, inner=P):
    """[K, M] -> [inner, K//inner, M] with the K axis innermost-major on
    partitions: out[p, o, m] = a[o*inner + p, m]."""
    k, m = a.shape
    return np.ascontiguousarray(a.reshape(k // inner, inner, m).transpose(1, 0, 2))


def kernel(x, Wg, W1, b1, W2, b2):
    x = np.asarray(x, dtype=np.float32)
    Wg = np.asarray(Wg, dtype=np.float32)
    W1 = np.asarray(W1, dtype=np.float32)
    b1 = np.asarray(b1, dtype=np.float32)
    W2 = np.asarray(W2, dtype=np.float32)
    b2 = np.asarray(b2, dtype=np.float32)
    assert x.shape == (B, S, D) and W1.shape == (E, F, D) and W2.shape == (E, D, F)

    x2d = x.reshape(N, D)

    # ---- host routing metadata (fp32, matches jax.lax.top_k tie-breaking) ----
    logits = x2d @ Wg.T  # [N, E]
    i1 = logits.argmax(1)
    masked = logits.copy()
    masked[np.arange(N), i1] = -np.inf
    i2 = masked.argmax(1)
    l1 = logits[np.arange(N), i1]
    l2 = logits[np.arange(N), i2]
    e2 = np.exp((l2 - l1).astype(np.float32))
    wt1 = (1.0 / (1.0 + e2)).astype(np.float32)
    wt2 = (e2 / (1.0 + e2)).astype(np.float32)

    tok_lists, wt_lists, counts = [], [], []
    for e in range(E):
        t_a = np.nonzero(i1 == e)[0]
        t_b = np.nonzero(i2 == e)[0]
        tok_lists.append(np.concatenate([t_a, t_b]))
        wt_lists.append(np.concatenate([wt1[t_a], wt2[t_b]]).astype(np.float32))
        counts.append(len(t_a) + len(t_b))
    max_load = max(counts)

    C = next((c for c in C_LADDER if c >= max_load), None)
    if C is None:
        C = ((max_load + P - 1) // P) * P
    nc = _get_nc(C)
    NTS = C // P

    # ---- per-core inputs ----
    WgT = Wg.T  # [D, E]
    wg_hi = WgT.astype(BF16)
    wg_lo = (WgT - wg_hi.astype(np.float32)).astype(BF16)
    wg_hi_l = _dp_major(wg_hi.astype(np.float32)).astype(BF16)
    wg_lo_l = _dp_major(wg_lo.astype(np.float32)).astype(BF16)

    in_maps = []
    for e in range(E):
        toks = np.zeros(C, dtype=np.int64)
        toks[: counts[e]] = tok_lists[e]
        wts = np.zeros(C, dtype=np.float32)
        wts[: counts[e]] = wt_lists[e]

        xpair = x2d[toks]  # [C, D]
        xp_l = _dp_major(xpair.T.astype(BF16).astype(np.float32)).astype(BF16)

        w1e = W1[e].reshape(F // P, P, D // P, P)  # [ft, fi, do, dp]
        w1t_l = (
            w1e.transpose(0, 3, 2, 1)  # [ft, dp, do, fi]
            .reshape(F // (2 * P), 2, P, D // P, P)  # [fp, j, dp, do, fi]
            .transpose(0, 2, 1, 3, 4)  # [fp, dp, j, do, fi]
        )
        w1t_l = np.ascontiguousarray(w1t_l).astype(BF16)
        w2e = W2[e].reshape(D // 256, 256, F // P, P)  # [dq, j, ft, fp]
        w2t_l = np.ascontiguousarray(w2e.transpose(0, 3, 2, 1)).astype(BF16)

        b1_l = np.ascontiguousarray(b1[e].reshape(F // P, P).T)  # [P, NF]
        b2b_l = np.ascontiguousarray(np.broadcast_to(b2[e], (P, D)))
        wp_l = np.ascontiguousarray(wts.reshape(NTS, P).T)  # [P, NTS]

        sl = slice(e * TOKENS_PER_CORE, (e + 1) * TOKENS_PER_CORE)
        xg = x2d[sl].T  # [D, TG] fp32
        xg_hi = xg.astype(BF16)
        xg_lo = (xg - xg_hi.astype(np.float32)).astype(BF16)
        xg_hi_l = _dp_major(xg_hi.astype(np.float32)).astype(BF16)
        xg_lo_l = _dp_major(xg_lo.astype(np.float32)).astype(BF16)

        in_maps.append(
            {
                "xp": xp_l,
                "w1t": w1t_l,
                "w2t": w2t_l,
                "b1c": b1_l,
                "b2b": b2b_l,
                "wp": wp_l,
                "xg_hi": xg_hi_l,
                "xg_lo": xg_lo_l,
                "wg_hi": wg_hi_l,
                "wg_lo": wg_lo_l,
            }
        )

    res = run_bass_kernel_spmd(nc, in_maps, core_ids=list(range(E)))

    # ---- host combine: unpermute pairs, sum the 2 pairs per token ----
    pair_out = np.concatenate(
        [res.results[e]["out"][: counts[e]] for e in range(E)], axis=0
    )  # [2N, D], already weighted + biased
    pair_tok = np.concatenate([tok_lists[e] for e in range(E)])
    order = np.argsort(pair_tok, kind="stable")
    out2d = pair_out[order].reshape(N, 2, D).sum(axis=1)

    # ---- aux loss from device partial sums ----
    stats = np.stack([res.results[e]["stats"][0] for e in range(E)])  # [E, 2E]
    load = stats[:, :E].sum(0) / N
    imp = stats[:, E:].sum(0) / N
    aux = np.float32(LB_COEF * E * np.sum(load * imp))

    return out2d.reshape(B, S, D).astype(np.float32), aux
